# revision 4
# baseline (speedup 1.0000x reference)
"""Contrastive-loss kernel for 8 TRN2 NeuronCores (Bass/Tile).

loss = sum_{i!=j}[ same(i,j)*d2(i,j) + diff(i,j)*relu(1-d(i,j))^2 ] / (n(n-1))

Decomposition:
  P = sum over same-label pairs of d2  ==  sum_c (2*n_c*S_c - 2*|M_c|^2)
      (exact identity; per-class count / sum |x|^2 / sum x computed on-device
       with bf16 matmuls over each core's row strip; sq carried as a bf16
       hi+lo pair so S_c keeps ~fp24 precision)
  Q = sum over diff-label pairs of relu(1-d)^2 -- nonzero only if some
      diff-label pair has d < 1 + slack.  The device certifies Q == 0 by
      screening every unordered pair once in a 22-dim random orthonormal
      projection y = bf16(Q22^T x):  ||y_i - y_j|| <= ||x_i - x_j|| + eps,
      so  d2_proj + 256*mask >= THETA=2  certifies  d >= 1.29 > 1.
      Screen matmul contraction K = 22 (proj) + 2 (sq/one) + 8 (mask) = 32
      rows -> four matmuls run CONCURRENTLY in the PE array via
      tile_position row-groups (4x throughput).  Row-tile g (128 rows)
      scans the cyclic column band [128g, 128g+4096) plus the
      distance-32 block [128g+4096, +128), PSUM tiles [128, 4x512];
      ScalarE consumes even tiles with Relu(t+THETA)+accum_out, VectorE
      reduce_max's odd tiles.  Any flag => exact host recompute of Q.

Per-core rhs is ROTATED by 1024*c cols so every core's band starts at
local col 0 (uniform DMA start, 5120-col rhs).  lhsT|rhs are packed in one
[32, 6144] DRAM tensor, DMA'd 4x to SBUF partition strips 0/32/64/96 for
the four PE row-groups.
"""

import numpy as np
import ml_dtypes

import concourse.bass as bass
import concourse.bacc as bacc
import concourse.tile as tile
from concourse import mybir
from concourse.bass_utils import run_bass_kernel_spmd

MARGIN = 1.0

N, D, NCLS, CORES = 8192, 64, 8, 8
ROWS = N // CORES            # rows per core
PT = 128                     # rows per row-tile
RT = ROWS // PT              # row-tiles per core
BAND = 4096                  # cyclic band columns per row-tile (dist 0..31)
EXT = 128                    # distance-32 extras block
RHSW = (RT - 1) * PT + BAND + EXT   # 896 + 4096 + 128 = 5120
KPROJ = 22                   # random-projection dims for the screen
KAUG = KPROJ + 2 + NCLS      # 32 = one PE row-group
BANDW = ROWS + RHSW          # 6144 cols: [lhsT | rhs]
MMN = 512                    # matmul free dim (one PSUM bank)
NGRP = 4                     # concurrent PE row-groups
WPR = BAND // (NGRP * MMN)   # waves per row-tile (2)
MSCALE = 16.0                # onehot scale; same-label mask adds 256
THETA = 2.0                  # flag threshold on projected d2
NCHUNK = ROWS // PT          # class-sum K chunks (8)
FDIM = D + 3                 # [x | sq_hi | sq_lo | 1] = 67
CLSW = NCLS + FDIM           # 75
NT = RT * WPR + 1            # consumer tiles: 16 band + 1 extras
PROJ_SEED = 42


def build_nc(repeats: int = 1):
    nc = bacc.Bacc("TRN2", target_bir_lowering=False, debug=False,
                   num_devices=CORES)
    bf16, f32 = mybir.dt.bfloat16, mybir.dt.float32

    band_d = nc.dram_tensor("band", [KAUG, BANDW], bf16, kind="ExternalInput")
    cls_d = nc.dram_tensor("clsin", [PT, NCHUNK, CLSW], bf16,
                           kind="ExternalInput")
    viol_d = nc.dram_tensor("viol", [PT, NT], f32, kind="ExternalOutput")
    clso_d = nc.dram_tensor("cls", [NCLS, FDIM], f32, kind="ExternalOutput")

    with tile.TileContext(nc) as tc:
        with (
            tc.tile_pool(name="w", bufs=1) as wpool,
            tc.tile_pool(name="ps", bufs=2, space="PSUM") as pspool,
            tc.tile_pool(name="scr", bufs=2) as scrpool,
            tc.tile_pool(name="acc", bufs=1) as accpool,
        ):
            clsT = wpool.tile([PT, NCHUNK, CLSW], bf16)
            nc.sync.dma_start(out=clsT[:], in_=cls_d[:])
            bandT = wpool.tile([NGRP * KAUG, BANDW], bf16)
            HEAD = 3072          # lhsT + first 2048 rhs cols
            for g in range(NGRP):
                nc.sync.dma_start(out=bandT[g * KAUG:(g + 1) * KAUG, 0:HEAD],
                                  in_=band_d[:, 0:HEAD])
            for g in range(NGRP):
                nc.sync.dma_start(
                    out=bandT[g * KAUG:(g + 1) * KAUG, HEAD:BANDW],
                    in_=band_d[:, HEAD:BANDW])

            viol_sb = accpool.tile([PT, NT], f32)
            cls_sb = accpool.tile([NCLS, FDIM], f32)
            theta_sb = accpool.tile([PT, 1], f32)
            nc.vector.memset(theta_sb, THETA)

            for _rep in range(repeats):
                # class sums first (small; overlaps the band DMA tail)
                psc = pspool.tile([PT, NGRP, MMN], f32, tag="ps")
                for i in range(NCHUNK):
                    nc.tensor.matmul(
                        psc[:NCLS, 0, 0:FDIM],
                        clsT[:, i, 0:NCLS],
                        clsT[:, i, NCLS:CLSW],
                        start=(i == 0), stop=(i == NCHUNK - 1))
                nc.scalar.copy(out=cls_sb[:], in_=psc[:NCLS, 0, 0:FDIM])

                # band screen: 2 waves of 4 concurrent row-group matmuls
                for r in range(RT):
                    for w in range(WPR):
                        ps = pspool.tile([PT, NGRP, MMN], f32, tag="ps")
                        for g in range(NGRP):
                            off = ROWS + r * PT + (w * NGRP + g) * MMN
                            nc.tensor.matmul(
                                ps[:, g, :],
                                bandT[g * KAUG:(g + 1) * KAUG,
                                      r * PT:(r + 1) * PT],
                                bandT[g * KAUG:(g + 1) * KAUG,
                                      off:off + MMN],
                                start=True, stop=True,
                                tile_position=(g * KAUG, 0))
                        t = r * WPR + w
                        if t % 2 == 0:   # ScalarE: sum of relu(t+theta)
                            scr = scrpool.tile([PT, NGRP * MMN], bf16,
                                               tag="scr")
                            nc.scalar.activation(
                                out=scr[:], in_=ps[:, :, :],
                                func=mybir.ActivationFunctionType.Relu,
                                bias=theta_sb[:], scale=1.0,
                                accum_out=viol_sb[:, t:t + 1])
                        else:            # VectorE: max of t
                            nc.vector.tensor_reduce(
                                out=viol_sb[:, t:t + 1], in_=ps[:, :, :],
                                axis=mybir.AxisListType.XY,
                                op=mybir.AluOpType.max)

                # distance-32 extras: 2 waves of 4 concurrent 128-col blocks
                pse = pspool.tile([PT, NGRP, MMN], f32, tag="ps")
                for e in range(2):
                    for g in range(NGRP):
                        r = e * NGRP + g
                        off = ROWS + r * PT + BAND
                        nc.tensor.matmul(
                            pse[:, g, e * EXT:(e + 1) * EXT],
                            bandT[g * KAUG:(g + 1) * KAUG,
                                  r * PT:(r + 1) * PT],
                            bandT[g * KAUG:(g + 1) * KAUG, off:off + EXT],
                            start=True, stop=True,
                            tile_position=(g * KAUG, 0))
                nc.vector.tensor_reduce(
                    out=viol_sb[:, NT - 1:NT], in_=pse[:, :, 0:2 * EXT],
                    axis=mybir.AxisListType.XY, op=mybir.AluOpType.max)

            nc.sync.dma_start(out=viol_d[:], in_=viol_sb[:])
            nc.sync.dma_start(out=clso_d[:], in_=cls_sb[:])
    nc.compile()
    return nc


def _proj_matrix() -> np.ndarray:
    """Fixed random orthonormal projection [D, KPROJ] (columns orthonormal),
    so ||Q^T v|| <= ||v|| for every v."""
    rng = np.random.default_rng(PROJ_SEED)
    a = rng.standard_normal((D, D))
    q, _ = np.linalg.qr(a)
    return np.ascontiguousarray(q[:, :KPROJ])


def prep_inputs(x: np.ndarray, label: np.ndarray):
    """Host-side sharding prep: bf16 screen matrices (rhs rotated per core)
    + bf16 class-sum operands."""
    x64 = x.astype(np.float64)
    q = _proj_matrix()
    y = (x64 @ q).astype(ml_dtypes.bfloat16)          # [N, KPROJ] bf16
    y64 = y.astype(np.float64)
    sqp = (y64 * y64).sum(axis=1)                     # exact ||y||^2
    shat = sqp.astype(ml_dtypes.bfloat16).astype(np.float64)

    oh = np.zeros((N, NCLS), np.float64)
    oh[np.arange(N), label] = 1.0

    lhst_all = np.concatenate(
        [y64, shat[:, None], np.ones((N, 1)), MSCALE * oh], axis=1
    ).T.astype(ml_dtypes.bfloat16)                    # [KAUG, N]
    rhs_all = np.concatenate(
        [2.0 * y64, -np.ones((N, 1)), -shat[:, None], -MSCALE * oh], axis=1
    ).T.astype(ml_dtypes.bfloat16)                    # [KAUG, N]
    rhs2 = np.concatenate([rhs_all, rhs_all], axis=1)  # for rotation

    sq = (x64 * x64).sum(axis=1)
    sq_hi = sq.astype(ml_dtypes.bfloat16).astype(np.float64)
    sq_lo = sq - sq_hi
    feat = np.concatenate(
        [oh, x64, sq_hi[:, None], sq_lo[:, None], np.ones((N, 1))], axis=1
    ).astype(ml_dtypes.bfloat16)                      # [N, CLSW]
    feat = feat.reshape(CORES, NCHUNK, PT, CLSW)
    feat = np.ascontiguousarray(feat.transpose(0, 2, 1, 3))

    in_maps = []
    for cc in range(CORES):
        band = np.concatenate(
            [lhst_all[:, cc * ROWS:(cc + 1) * ROWS],
             rhs2[:, cc * ROWS:cc * ROWS + RHSW]], axis=1)
        in_maps.append({
            "band": np.ascontiguousarray(band),
            "clsin": feat[cc],
        })
    return in_maps


def _exact_q(x: np.ndarray, label: np.ndarray) -> float:
    """Exact Q = sum over ordered diff-label pairs of relu(1-d)^2 (fp64,
    chunked).  Only runs when the device flags a potential margin pair."""
    x64 = x.astype(np.float64)
    sq = (x64 * x64).sum(axis=1)
    q = 0.0
    step = 1024
    for a in range(0, N, step):
        d2 = sq[a:a + step, None] + sq[None, :] - 2.0 * (x64[a:a + step] @ x64.T)
        d = np.sqrt(np.maximum(d2, 0.0))
        diff = label[a:a + step, None] != label[None, :]
        r = np.maximum(MARGIN - d, 0.0)
        offdiag = np.arange(a, a + step)[:, None] != np.arange(N)[None, :]
        q += float((r * r)[diff & offdiag].sum())
    return q


def finish(results, x: np.ndarray, label: np.ndarray) -> np.float32:
    cls = np.zeros((NCLS, FDIM), np.float64)
    for rr in results:
        cls += rr["cls"].astype(np.float64)
    M = cls[:, :D]
    S = cls[:, D] + cls[:, D + 1]
    ncnt = cls[:, D + 2]
    P = float((2.0 * ncnt * S - 2.0 * (M * M).sum(axis=1)).sum())

    flagged = False
    for rr in results:
        v = rr["viol"]
        if (v[:, 0:NT - 1:2] > 0.0).any():             # ACT relu sums
            flagged = True
        if (v[:, 1:NT - 1:2] > -THETA).any():          # DVE maxes
            flagged = True
        if (v[:, NT - 1] > -THETA).any():              # extras max
            flagged = True
    Q = _exact_q(x, label) if flagged else 0.0

    return np.float32((P + Q) / (N * (N - 1)))


_NC_CACHE: dict = {}


def kernel(output: np.ndarray, label: np.ndarray) -> np.ndarray:
    x = np.asarray(output, dtype=np.float32)
    lab = np.asarray(label).astype(np.int64)
    assert x.shape == (N, D) and lab.shape == (N,)

    if "nc" not in _NC_CACHE:
        _NC_CACHE["nc"] = build_nc()
    nc = _NC_CACHE["nc"]

    in_maps = prep_inputs(x, lab)
    res = run_bass_kernel_spmd(nc, in_maps, core_ids=list(range(CORES)))
    loss = finish(res.results, x, lab)
    return np.asarray(loss, dtype=np.float32)


# revision 6
# speedup vs baseline: 1.8349x; 1.8349x over previous
"""Contrastive-loss kernel for 8 TRN2 NeuronCores (Bass/Tile).

loss = sum_{i!=j}[ same(i,j)*d2(i,j) + diff(i,j)*relu(1-d(i,j))^2 ] / (n(n-1))

Decomposition:
  P = sum over same-label pairs of d2  ==  sum_c (2*n_c*S_c - 2*|M_c|^2)
      (exact identity; per-class count / sum |x|^2 / sum x computed on-device
       with bf16 matmuls over each core's row strip; sq carried as a bf16
       hi+lo pair so S_c keeps ~fp24 precision)
  Q = sum over diff-label pairs of relu(1-d)^2 -- nonzero only if some
      diff-label pair has d < 1 + slack.  The device certifies Q == 0 by
      screening every unordered pair once in a 22-dim random orthonormal
      projection y = bf16(Q22^T x):  ||y_i - y_j|| <= ||x_i - x_j|| + eps,
      so  d2_proj + 256*mask >= THETA=2  certifies  d >= 1.29 > 1.
      Screen matmul contraction K = 22 (proj) + 2 (sq/one) + 8 (mask) = 32
      rows -> four matmuls run CONCURRENTLY in the PE array via
      tile_position row-groups (4x throughput).  Row-tile g (128 rows)
      scans the cyclic column band [128g, 128g+4096) plus the
      distance-32 block [128g+4096, +128), PSUM tiles [128, 4x512];
      ScalarE consumes even tiles with Relu(t+THETA)+accum_out, VectorE
      reduce_max's odd tiles.  Any flag => exact host recompute of Q.

Per-core rhs is ROTATED by 1024*c cols so every core's band starts at
local col 0 (uniform DMA start, 5120-col rhs).  lhsT|rhs are packed in one
[32, 6144] DRAM tensor, DMA'd 4x to SBUF partition strips 0/32/64/96 for
the four PE row-groups.
"""

import numpy as np
import ml_dtypes

import concourse.bass as bass
import concourse.bacc as bacc
import concourse.tile as tile
from concourse import mybir
from concourse.bass_utils import run_bass_kernel_spmd

MARGIN = 1.0

N, D, NCLS, CORES = 8192, 64, 8, 8
ROWS = N // CORES            # rows per core
PT = 128                     # rows per row-tile
RT = ROWS // PT              # row-tiles per core
BAND = 4096                  # cyclic band columns per row-tile (dist 0..31)
EXT = 128                    # distance-32 extras block
RHSW = (RT - 1) * PT + BAND + EXT   # 896 + 4096 + 128 = 5120
KPROJ = 22                   # random-projection dims for the screen
KAUG = KPROJ + 2 + NCLS      # 32 = one PE row-group
BANDW = ROWS + RHSW          # 6144 cols: [lhsT | rhs]
MMN = 512                    # matmul free dim (one PSUM bank)
NGRP = 4                     # concurrent PE row-groups
WPR = BAND // (NGRP * MMN)   # waves per row-tile (2)
MSCALE = 16.0                # onehot scale; same-label mask adds 256
THETA = 2.0                  # flag threshold on projected d2
NCHUNK = ROWS // PT          # class-sum K chunks (8)
FDIM = D + 3                 # [x | sq_hi | sq_lo | 1] = 67
CLSW = NCLS + FDIM           # 75
NT = RT * WPR + 1            # consumer tiles: 16 band + 1 extras
PROJ_SEED = 42


def build_nc(repeats: int = 1):
    nc = bacc.Bacc("TRN2", target_bir_lowering=False, debug=False,
                   num_devices=CORES)
    bf16, f32 = mybir.dt.bfloat16, mybir.dt.float32

    band_d = nc.dram_tensor("band", [NGRP * KAUG, BANDW], bf16,
                            kind="ExternalInput")
    cls_d = nc.dram_tensor("clsin", [PT, NCHUNK, CLSW], bf16,
                           kind="ExternalInput")
    viol_d = nc.dram_tensor("viol", [PT, NT], f32, kind="ExternalOutput")
    clso_d = nc.dram_tensor("cls", [NCLS, FDIM], f32, kind="ExternalOutput")

    # w0/g3/r0 needs cols < 3072; w0 of r1..7 needs < 3968; rest in tail
    HEAD1, HEAD2 = ROWS + 2 * NGRP * MMN // 2, ROWS + NGRP * MMN + (RT - 1) * PT

    with tile.TileContext(nc) as tc:
        with (
            tc.tile_pool(name="w", bufs=1) as wpool,
            tc.tile_pool(name="ps", bufs=2, space="PSUM") as pspool,
            tc.tile_pool(name="scr", bufs=2) as scrpool,
            tc.tile_pool(name="acc", bufs=1) as accpool,
        ):
            clsT = wpool.tile([PT, NCHUNK, CLSW], bf16)
            nc.scalar.dma_start(out=clsT[:], in_=cls_d[:])
            bandT = wpool.tile([NGRP * KAUG, BANDW], bf16)
            nc.sync.dma_start(out=bandT[:, 0:HEAD1], in_=band_d[:, 0:HEAD1])
            nc.sync.dma_start(out=bandT[:, HEAD1:HEAD2],
                              in_=band_d[:, HEAD1:HEAD2])
            nc.sync.dma_start(out=bandT[:, HEAD2:BANDW],
                              in_=band_d[:, HEAD2:BANDW])

            viol_sb = accpool.tile([PT, NT], f32)
            cls_sb = accpool.tile([NCLS, FDIM], f32)
            theta_sb = accpool.tile([PT, 1], f32)
            nc.vector.memset(theta_sb, THETA)

            for _rep in range(repeats):
                # class sums first (small; overlaps the band DMA tail and
                # starts the PE HAM warm-up)
                psc = pspool.tile([PT, NGRP, MMN], f32, tag="ps")
                for i in range(NCHUNK):
                    nc.tensor.matmul(
                        psc[:NCLS, 0, 0:FDIM],
                        clsT[:, i, 0:NCLS],
                        clsT[:, i, NCLS:CLSW],
                        start=(i == 0), stop=(i == NCHUNK - 1))
                nc.scalar.copy(out=cls_sb[:], in_=psc[:NCLS, 0, 0:FDIM])

                # band screen: w-major waves of 4 concurrent row-group MMs
                for w in range(WPR):
                    for r in range(RT):
                        ps = pspool.tile([PT, NGRP, MMN], f32, tag="ps")
                        for g in range(NGRP):
                            off = ROWS + r * PT + (w * NGRP + g) * MMN
                            nc.tensor.matmul(
                                ps[:, g, :],
                                bandT[g * KAUG:(g + 1) * KAUG,
                                      r * PT:(r + 1) * PT],
                                bandT[g * KAUG:(g + 1) * KAUG,
                                      off:off + MMN],
                                start=True, stop=True,
                                tile_position=(g * KAUG, 0))
                        t = w * RT + r
                        if t % 2 == 0:   # ScalarE: sum of relu(t+theta)
                            scr = scrpool.tile([PT, NGRP * MMN], bf16,
                                               tag="scr")
                            nc.scalar.activation(
                                out=scr[:], in_=ps[:, :, :],
                                func=mybir.ActivationFunctionType.Relu,
                                bias=theta_sb[:], scale=1.0,
                                accum_out=viol_sb[:, t:t + 1])
                        else:            # VectorE: max of t
                            nc.vector.tensor_reduce(
                                out=viol_sb[:, t:t + 1], in_=ps[:, :, :],
                                axis=mybir.AxisListType.XY,
                                op=mybir.AluOpType.max)

                # distance-32 extras: 2 waves of 4 concurrent 128-col blocks
                pse = pspool.tile([PT, NGRP, MMN], f32, tag="ps")
                for e in range(2):
                    for g in range(NGRP):
                        r = e * NGRP + g
                        off = ROWS + r * PT + BAND
                        nc.tensor.matmul(
                            pse[:, g, e * EXT:(e + 1) * EXT],
                            bandT[g * KAUG:(g + 1) * KAUG,
                                  r * PT:(r + 1) * PT],
                            bandT[g * KAUG:(g + 1) * KAUG, off:off + EXT],
                            start=True, stop=True,
                            tile_position=(g * KAUG, 0))
                nc.vector.tensor_reduce(
                    out=viol_sb[:, NT - 1:NT], in_=pse[:, :, 0:2 * EXT],
                    axis=mybir.AxisListType.XY, op=mybir.AluOpType.max)

            nc.sync.dma_start(out=viol_d[:], in_=viol_sb[:])
            nc.sync.dma_start(out=clso_d[:], in_=cls_sb[:])
    nc.compile()
    return nc


def _proj_matrix() -> np.ndarray:
    """Fixed random orthonormal projection [D, KPROJ] (columns orthonormal),
    so ||Q^T v|| <= ||v|| for every v."""
    rng = np.random.default_rng(PROJ_SEED)
    a = rng.standard_normal((D, D))
    q, _ = np.linalg.qr(a)
    return np.ascontiguousarray(q[:, :KPROJ])


def prep_inputs(x: np.ndarray, label: np.ndarray):
    """Host-side sharding prep: bf16 screen matrices (rhs rotated per core)
    + bf16 class-sum operands."""
    x64 = x.astype(np.float64)
    q = _proj_matrix()
    y = (x64 @ q).astype(ml_dtypes.bfloat16)          # [N, KPROJ] bf16
    y64 = y.astype(np.float64)
    sqp = (y64 * y64).sum(axis=1)                     # exact ||y||^2
    shat = sqp.astype(ml_dtypes.bfloat16).astype(np.float64)

    oh = np.zeros((N, NCLS), np.float64)
    oh[np.arange(N), label] = 1.0

    lhst_all = np.concatenate(
        [y64, shat[:, None], np.ones((N, 1)), MSCALE * oh], axis=1
    ).T.astype(ml_dtypes.bfloat16)                    # [KAUG, N]
    rhs_all = np.concatenate(
        [2.0 * y64, -np.ones((N, 1)), -shat[:, None], -MSCALE * oh], axis=1
    ).T.astype(ml_dtypes.bfloat16)                    # [KAUG, N]
    rhs2 = np.concatenate([rhs_all, rhs_all], axis=1)  # for rotation

    sq = (x64 * x64).sum(axis=1)
    sq_hi = sq.astype(ml_dtypes.bfloat16).astype(np.float64)
    sq_lo = sq - sq_hi
    feat = np.concatenate(
        [oh, x64, sq_hi[:, None], sq_lo[:, None], np.ones((N, 1))], axis=1
    ).astype(ml_dtypes.bfloat16)                      # [N, CLSW]
    feat = feat.reshape(CORES, NCHUNK, PT, CLSW)
    feat = np.ascontiguousarray(feat.transpose(0, 2, 1, 3))

    in_maps = []
    for cc in range(CORES):
        band = np.concatenate(
            [lhst_all[:, cc * ROWS:(cc + 1) * ROWS],
             rhs2[:, cc * ROWS:cc * ROWS + RHSW]], axis=1)
        in_maps.append({
            "band": np.ascontiguousarray(np.tile(band, (NGRP, 1))),
            "clsin": feat[cc],
        })
    return in_maps


def _exact_q(x: np.ndarray, label: np.ndarray) -> float:
    """Exact Q = sum over ordered diff-label pairs of relu(1-d)^2 (fp64,
    chunked).  Only runs when the device flags a potential margin pair."""
    x64 = x.astype(np.float64)
    sq = (x64 * x64).sum(axis=1)
    q = 0.0
    step = 1024
    for a in range(0, N, step):
        d2 = sq[a:a + step, None] + sq[None, :] - 2.0 * (x64[a:a + step] @ x64.T)
        d = np.sqrt(np.maximum(d2, 0.0))
        diff = label[a:a + step, None] != label[None, :]
        r = np.maximum(MARGIN - d, 0.0)
        offdiag = np.arange(a, a + step)[:, None] != np.arange(N)[None, :]
        q += float((r * r)[diff & offdiag].sum())
    return q


def finish(results, x: np.ndarray, label: np.ndarray) -> np.float32:
    cls = np.zeros((NCLS, FDIM), np.float64)
    for rr in results:
        cls += rr["cls"].astype(np.float64)
    M = cls[:, :D]
    S = cls[:, D] + cls[:, D + 1]
    ncnt = cls[:, D + 2]
    P = float((2.0 * ncnt * S - 2.0 * (M * M).sum(axis=1)).sum())

    flagged = False
    for rr in results:
        v = rr["viol"]
        if (v[:, 0:NT - 1:2] > 0.0).any():             # ACT relu sums
            flagged = True
        if (v[:, 1:NT - 1:2] > -THETA).any():          # DVE maxes
            flagged = True
        if (v[:, NT - 1] > -THETA).any():              # extras max
            flagged = True
    Q = _exact_q(x, label) if flagged else 0.0

    return np.float32((P + Q) / (N * (N - 1)))


_NC_CACHE: dict = {}


def kernel(output: np.ndarray, label: np.ndarray) -> np.ndarray:
    x = np.asarray(output, dtype=np.float32)
    lab = np.asarray(label).astype(np.int64)
    assert x.shape == (N, D) and lab.shape == (N,)

    if "nc" not in _NC_CACHE:
        _NC_CACHE["nc"] = build_nc()
    nc = _NC_CACHE["nc"]

    in_maps = prep_inputs(x, lab)
    res = run_bass_kernel_spmd(nc, in_maps, core_ids=list(range(CORES)))
    loss = finish(res.results, x, lab)
    return np.asarray(loss, dtype=np.float32)
